# revision 26
# baseline (speedup 1.0000x reference)
import sys
import contextlib

sys.path.insert(0, "/opt/trn_rl_repo")

import numpy as np

import concourse.bass as bass
import concourse.mybir as mybir
import concourse.tile as tile
from concourse import bacc
from concourse.bass_utils import run_bass_kernel_spmd

# Problem constants (nn_DT_GCN_Lite): hardcoded per harness contract.
N_NODES = 100000
N_EDGES = 1000000
IN_CH = 64
OUT_CH = 128
N_CORES = 8

WINDOW = 128                       # nodes per destination window (= PSUM tile width)
WINDOWS_PER_CORE = 98              # 98 * 128 = 12544 nodes per core
NODES_PER_CORE = WINDOWS_PER_CORE * WINDOW
N_NODES_PAD = NODES_PER_CORE * N_CORES  # 100352

P = 128                            # edges per block (one partition each)
CHUNK = 25000                      # nodes per x-chunk (<=32768 for int16)
N_CHUNKS = 4                       # 4 * 25000 = 100000, balanced buckets
G = 7                              # windows per gather group (98 = 14 * 7)

FP = mybir.dt.float32
BF = mybir.dt.bfloat16
NP_FP = np.float32


def _layout(cap_wc):
    """Static layout from cap_wc [W, C] (padded edge counts per bucket,
    multiples of 128). Returns dict of offsets."""
    W = cap_wc.shape[0]
    nblk_wc = cap_wc // P                               # [W, C]
    nblk_w = nblk_wc.sum(axis=1)                        # blocks per window
    NBLK = int(nblk_wc.sum())

    # rowl/wts block columns: (w, ch, j) order
    blk_off_wc = np.zeros((W, N_CHUNKS), dtype=np.int64)
    acc = 0
    for w in range(W):
        for ch in range(N_CHUNKS):
            blk_off_wc[w, ch] = acc
            acc += nblk_wc[w, ch]

    groups = [list(range(g * G, (g + 1) * G)) for g in range(W // G)]
    # msg columns within each group: (ch, w, j) order; gathers are per (g, ch)
    msg_off_gchw = {}
    msg_cols_g = []
    gth = []  # per sub-gather: (gi, ch, ni, sidx_off, moff)
    MAX_NI = 896                       # ring limit: <=1008 idxs and multiple of 128
    sidx = 0
    for gi, ws in enumerate(groups):
        off = 0
        for ch in range(N_CHUNKS):
            ni = int(cap_wc[ws, ch].sum())
            done = 0
            while done < ni:
                sub = min(MAX_NI, ni - done)
                gth.append((gi, ch, sub, sidx, off + done // P))
                sidx += sub // 16
                done += sub
            for w in ws:
                msg_off_gchw[(w, ch)] = off
                off += int(nblk_wc[w, ch])
        msg_cols_g.append(off)
    return dict(
        nblk_wc=nblk_wc, nblk_w=nblk_w, NBLK=NBLK, blk_off_wc=blk_off_wc,
        groups=groups, msg_off_gchw=msg_off_gchw, msg_cols_g=msg_cols_g,
        gth=gth, SIDX=sidx,
    )


def build_nc(cap_wc, repeat=1):
    L = _layout(cap_wc)
    NBLK, SIDX = L["NBLK"], L["SIDX"]
    nc = bacc.Bacc("TRN2", target_bir_lowering=False, num_swdge_queues=4)

    x_d = nc.dram_tensor("x", [N_NODES, IN_CH], FP, kind="ExternalInput")
    idx_d = nc.dram_tensor("idx16", [P, SIDX], mybir.dt.int16, kind="ExternalInput")
    rowl_d = nc.dram_tensor("rowl", [P, NBLK], FP, kind="ExternalInput")
    wtsm_d = nc.dram_tensor("wtsm", [P, NBLK], FP, kind="ExternalInput")
    wt_d = nc.dram_tensor("wt", [IN_CH, OUT_CH], FP, kind="ExternalInput")
    bias_d = nc.dram_tensor("bias", [1, OUT_CH], FP, kind="ExternalInput")
    out_d = nc.dram_tensor("out", [NODES_PER_CORE, OUT_CH], FP, kind="ExternalOutput")

    max_msg_cols = max(L["msg_cols_g"])
    max_nblk_w = int(L["nblk_w"].max())

    with tile.TileContext(nc) as tc:
        with (
            tc.tile_pool(name="const", bufs=1) as const_pool,
            tc.tile_pool(name="msg", bufs=4) as msg_pool,
            tc.tile_pool(name="msgb", bufs=4) as msgb_pool,
            tc.tile_pool(name="oh", bufs=8) as oh_pool,
            tc.tile_pool(name="aggp", bufs=2, space="PSUM") as aggp_pool,
            tc.tile_pool(name="aggs", bufs=3) as aggs_pool,
            tc.tile_pool(name="outp", bufs=2, space="PSUM") as outp_pool,
            tc.tile_pool(name="outs", bufs=3) as outs_pool,
        ):
            idx_sb = const_pool.tile([P, SIDX], mybir.dt.int16)
            rowl_sb = const_pool.tile([P, NBLK], FP)
            rowl_bf = const_pool.tile([P, NBLK], BF)
            wtsm_sb = const_pool.tile([P, NBLK], FP)
            iota_sb = const_pool.tile([P, WINDOW], FP)
            iota_bf = const_pool.tile([P, WINDOW], BF)
            wt_sb = const_pool.tile([IN_CH, OUT_CH], FP)
            wt_bf = const_pool.tile([IN_CH, OUT_CH], BF)
            bias_sb = const_pool.tile([1, OUT_CH], FP)
            bias_bf = const_pool.tile([1, OUT_CH], BF)
            ones_bf = const_pool.tile([1, WINDOW], BF)

            nc.sync.dma_start(idx_sb[:], idx_d[:])
            nc.sync.dma_start(rowl_sb[:], rowl_d[:])
            nc.sync.dma_start(wtsm_sb[:], wtsm_d[:])
            nc.sync.dma_start(wt_sb[:], wt_d[:])
            nc.sync.dma_start(bias_sb[:], bias_d[:])
            nc.gpsimd.iota(
                iota_sb[:], pattern=[[1, WINDOW]], base=0,
                channel_multiplier=0, allow_small_or_imprecise_dtypes=True,
            )
            nc.vector.tensor_copy(iota_bf[:], iota_sb[:])
            nc.vector.tensor_copy(rowl_bf[:], rowl_sb[:])
            nc.vector.tensor_copy(wt_bf[:], wt_sb[:])
            nc.vector.tensor_copy(bias_bf[:], bias_sb[:])
            nc.vector.memset(ones_bf[:], 1.0)

            loop_cm = tc.For_i(0, repeat, 1) if repeat > 1 else contextlib.nullcontext()
            with loop_cm:
                for gi, ws in enumerate(L["groups"]):
                    mcols = L["msg_cols_g"][gi]
                    msg = msg_pool.tile([P, max_msg_cols * IN_CH], FP, tag="msg")
                    msgb = msgb_pool.tile([P, max_msg_cols * IN_CH], BF, tag="msgb")
                    # sub-gathers (ring-limited) per (group, chunk)
                    for (gi2, ch, ni, sidx_off, moff) in L["gth"]:
                        if gi2 != gi or ni == 0:
                            continue
                        nblk_g = ni // P
                        nc.gpsimd.dma_gather(
                            out_ap=msg[:, moff * IN_CH : (moff + nblk_g) * IN_CH]
                            .rearrange("p (k d) -> p k d", k=nblk_g),
                            in_ap=x_d[ch * CHUNK : min((ch + 1) * CHUNK, N_NODES), :],
                            idxs_ap=idx_sb[:, sidx_off : sidx_off + ni // 16],
                            num_idxs=ni,
                            num_idxs_reg=ni,
                            elem_size=IN_CH,
                            queue_num=(sidx_off // 56) % 4,
                        )
                    # msgb = msg * w -> bf16 (one batched op per (group, chunk)
                    # region so early chunks feed PE while later chunks gather)
                    gbase = sum(L["msg_cols_g"][:gi])
                    for ch in range(N_CHUNKS):
                        c0 = L["msg_off_gchw"][(ws[0], ch)]
                        ncols = int(sum(L["nblk_wc"][w, ch] for w in ws))
                        if ncols == 0:
                            continue
                        wts_ap = wtsm_sb[:, gbase + c0 : gbase + c0 + ncols]
                        nc.vector.tensor_tensor(
                            out=msgb[:, c0 * IN_CH : (c0 + ncols) * IN_CH]
                            .rearrange("p (k d) -> p k d", k=ncols),
                            in0=msg[:, c0 * IN_CH : (c0 + ncols) * IN_CH]
                            .rearrange("p (k d) -> p k d", k=ncols),
                            in1=bass.AP(
                                wts_ap.tensor, wts_ap.offset,
                                [wts_ap.ap[0], [wts_ap.ap[1][0], ncols], [0, IN_CH]],
                            ),
                            op=mybir.AluOpType.mult,
                        )
                    for w in ws:
                        nblkw = int(L["nblk_w"][w])
                        b0 = int(L["blk_off_wc"][w, 0])
                        # one-hot: eq(iota, rowl_bcast), batched per window, bf16
                        oh = oh_pool.tile([P, max_nblk_w * WINDOW], BF, tag="oh")
                        rowl_ap = rowl_bf[:, b0 : b0 + nblkw]
                        iap = iota_bf[:]
                        nc.vector.tensor_tensor(
                            out=oh[:, : nblkw * WINDOW].rearrange("p (k d) -> p k d", k=nblkw),
                            in0=bass.AP(
                                iap.tensor, iap.offset,
                                [iap.ap[0], [0, nblkw], [iap.ap[1][0], WINDOW]],
                            ),
                            in1=bass.AP(
                                rowl_ap.tensor, rowl_ap.offset,
                                [rowl_ap.ap[0], [rowl_ap.ap[1][0], nblkw], [0, WINDOW]],
                            ),
                            op=mybir.AluOpType.is_equal,
                        )
                        aggT = aggp_pool.tile([IN_CH, WINDOW], FP)
                        jj = 0
                        for ch in range(N_CHUNKS):
                            nblk_c = int(L["nblk_wc"][w, ch])
                            moff = L["msg_off_gchw"][(w, ch)]
                            for j in range(nblk_c):
                                nc.tensor.matmul(
                                    aggT[:],
                                    lhsT=msgb[:, (moff + j) * IN_CH : (moff + j + 1) * IN_CH],
                                    rhs=oh[:, jj * WINDOW : (jj + 1) * WINDOW],
                                    start=(jj == 0),
                                    stop=(jj == nblkw - 1),
                                )
                                jj += 1
                        aggT_sb = aggs_pool.tile([IN_CH, WINDOW], BF)
                        nc.vector.tensor_copy(aggT_sb[:], aggT[:])
                        op = outp_pool.tile([WINDOW, OUT_CH], FP)
                        nc.tensor.matmul(op[:], lhsT=aggT_sb[:], rhs=wt_bf[:], start=True, stop=False)
                        nc.tensor.matmul(op[:], lhsT=ones_bf[:], rhs=bias_bf[:], start=False, stop=True)
                        out_sb = outs_pool.tile([WINDOW, OUT_CH], FP)
                        nc.vector.tensor_copy(out_sb[:], op[:])
                        nc.sync.dma_start(out_d[w * WINDOW : (w + 1) * WINDOW, :], out_sb[:])
    nc.compile()
    return nc


def preprocess(x, edge_index, edge_weight):
    """Bucket edges by (core, window, chunk); pad each bucket to a multiple of
    P (shared across cores). Returns per-core input maps and cap_wc."""
    row = np.asarray(edge_index[0], dtype=np.int64)
    col = np.asarray(edge_index[1], dtype=np.int64)
    wts = np.asarray(edge_weight, dtype=NP_FP)

    gwin = row >> 7                                   # global window id
    ch = col // CHUNK
    key = gwin * N_CHUNKS + ch
    order = np.argsort(key, kind="stable")
    row_s, col_s, w_s, key_s = row[order], col[order], wts[order], key[order]

    n_keys = (N_NODES_PAD >> 7) * N_CHUNKS
    counts = np.bincount(key_s, minlength=n_keys).reshape(
        N_CORES, WINDOWS_PER_CORE, N_CHUNKS
    )
    starts = np.zeros(n_keys + 1, dtype=np.int64)
    np.cumsum(counts.reshape(-1), out=starts[1:])

    cap_wc = -(-counts.max(axis=0) // P) * P          # [W, C] padded counts
    for w in range(WINDOWS_PER_CORE):
        if cap_wc[w].sum() == 0:
            cap_wc[w, 0] = P

    L = _layout(cap_wc)
    NBLK, SIDX = L["NBLK"], L["SIDX"]
    nblk_wc = L["nblk_wc"]

    in_maps = []
    for c in range(N_CORES):
        rowl_a = np.zeros((P, NBLK), dtype=NP_FP)
        wts_blk = np.zeros((P, NBLK), dtype=NP_FP)   # (w, ch, j) block order
        idx_blk = np.zeros((P, NBLK), dtype=np.int16)  # local idx, same order
        for w in range(WINDOWS_PER_CORE):
            for chn in range(N_CHUNKS):
                nblk = int(nblk_wc[w, chn])
                if nblk == 0:
                    continue
                g = c * WINDOWS_PER_CORE + w
                k = g * N_CHUNKS + chn
                s, e = starts[k], starts[k + 1]
                cnt = e - s
                cap = nblk * P
                b0 = int(L["blk_off_wc"][w, chn])
                re_ = np.zeros((cap,), dtype=NP_FP)
                we = np.zeros((cap,), dtype=NP_FP)
                ce = np.zeros((cap,), dtype=np.int16)
                re_[:cnt] = (row_s[s:e] - g * WINDOW).astype(NP_FP)
                we[:cnt] = w_s[s:e]
                ce[:cnt] = (col_s[s:e] - chn * CHUNK).astype(np.int16)
                # edge i -> partition i%128, block i//128
                rowl_a[:, b0 : b0 + nblk] = re_.reshape(nblk, P).T
                wts_blk[:, b0 : b0 + nblk] = we.reshape(nblk, P).T
                idx_blk[:, b0 : b0 + nblk] = ce.reshape(nblk, P).T

        # wtsm: weights in msg-column order ((g, ch, w, j))
        wtsm_a = np.zeros((P, NBLK), dtype=NP_FP)
        # idx16: wrapped indices per (g, ch) gather stream
        idx16 = np.zeros((P, SIDX), dtype=np.int16)
        emitted = set()
        for (gi, chn, ni, sidx_off, moff) in L["gth"]:
            if ni == 0 or (gi, chn) in emitted:
                continue
            emitted.add((gi, chn))
            ni = int(cap_wc[L["groups"][gi], chn].sum())  # full (g,ch) stream
            ws = L["groups"][gi]
            gbase = sum(L["msg_cols_g"][:gi])
            stream = np.zeros((ni,), dtype=np.int16)
            soff = 0
            for w in ws:
                nblk = int(nblk_wc[w, chn])
                if nblk == 0:
                    continue
                b0 = int(L["blk_off_wc"][w, chn])
                cap = nblk * P
                # edges of this bucket in (block, partition) order
                ed_idx = idx_blk[:, b0 : b0 + nblk].T.reshape(-1)
                ed_wts = wts_blk[:, b0 : b0 + nblk].T.reshape(-1)
                stream[soff : soff + cap] = ed_idx
                mo = gbase + L["msg_off_gchw"][(w, chn)]
                wtsm_a[:, mo : mo + nblk] = ed_wts.reshape(nblk, P).T
                soff += cap
            # wrap: idx i -> partition i%16, column i//16; replicate 8x
            wrapped = stream.reshape(ni // 16, 16).T    # [16, ni/16]
            idx16[:, sidx_off : sidx_off + ni // 16] = np.tile(wrapped, (8, 1))

        in_maps.append({"idx16": idx16, "rowl": rowl_a, "wtsm": wtsm_a})
    return in_maps, cap_wc


_CACHE = {}


def finalize_in_maps(in_maps, x, W, b):
    """Attach shared tensors (x, weights, bias)."""
    x = np.asarray(x, dtype=NP_FP)
    wt = np.ascontiguousarray(np.asarray(W, dtype=NP_FP).T)
    bias = np.asarray(b, dtype=NP_FP).reshape(1, OUT_CH)
    for c in range(N_CORES):
        in_maps[c]["x"] = x
        in_maps[c]["wt"] = wt
        in_maps[c]["bias"] = bias
    return in_maps


def kernel(x, edge_index, edge_weight, W, b):
    x = np.asarray(x, dtype=NP_FP)

    in_maps, cap_wc = preprocess(x, edge_index, edge_weight)

    key = cap_wc.tobytes()
    if key not in _CACHE:
        _CACHE[key] = build_nc(cap_wc)
    nc = _CACHE[key]

    finalize_in_maps(in_maps, x, W, b)

    res = run_bass_kernel_spmd(nc, in_maps, core_ids=list(range(N_CORES)))
    out = np.concatenate([res.results[c]["out"] for c in range(N_CORES)], axis=0)
    return out[:N_NODES]


# revision 28
# speedup vs baseline: 1.0114x; 1.0114x over previous
import sys
import contextlib

sys.path.insert(0, "/opt/trn_rl_repo")

import numpy as np

import concourse.bass as bass
import concourse.mybir as mybir
import concourse.tile as tile
from concourse import bacc
from concourse.bass_utils import run_bass_kernel_spmd

# Problem constants (nn_DT_GCN_Lite): hardcoded per harness contract.
N_NODES = 100000
N_EDGES = 1000000
IN_CH = 64
OUT_CH = 128
N_CORES = 8

WINDOW = 128                       # nodes per destination window (= PSUM tile width)
WINDOWS_PER_CORE = 98              # 98 * 128 = 12544 nodes per core
NODES_PER_CORE = WINDOWS_PER_CORE * WINDOW
N_NODES_PAD = NODES_PER_CORE * N_CORES  # 100352

P = 128                            # edges per block (one partition each)
CHUNK = 25000                      # nodes per x-chunk (<=32768 for int16)
N_CHUNKS = 4                       # 4 * 25000 = 100000, balanced buckets
G = 7                              # windows per gather group (98 = 14 * 7)

FP = mybir.dt.float32
BF = mybir.dt.bfloat16
NP_FP = np.float32


def _layout(cap_wc):
    """Static layout from cap_wc [W, C] (padded edge counts per bucket,
    multiples of 128). Returns dict of offsets."""
    W = cap_wc.shape[0]
    nblk_wc = cap_wc // P                               # [W, C]
    nblk_w = nblk_wc.sum(axis=1)                        # blocks per window
    NBLK = int(nblk_wc.sum())

    # rowl/wts block columns: (w, ch, j) order
    blk_off_wc = np.zeros((W, N_CHUNKS), dtype=np.int64)
    acc = 0
    for w in range(W):
        for ch in range(N_CHUNKS):
            blk_off_wc[w, ch] = acc
            acc += nblk_wc[w, ch]

    groups = [list(range(g * G, (g + 1) * G)) for g in range(W // G)]
    # msg columns within each group: (ch, w, j) order; gathers are per (g, ch)
    msg_off_gchw = {}
    msg_cols_g = []
    gth = []  # per sub-gather: (gi, ch, ni, sidx_off, moff)
    MAX_NI = 896                       # ring limit: <=1008 idxs and multiple of 128
    sidx = 0
    for gi, ws in enumerate(groups):
        off = 0
        for ch in range(N_CHUNKS):
            ni = int(cap_wc[ws, ch].sum())
            done = 0
            while done < ni:
                sub = min(MAX_NI, ni - done)
                gth.append((gi, ch, sub, sidx, off + done // P))
                sidx += sub // 16
                done += sub
            for w in ws:
                msg_off_gchw[(w, ch)] = off
                off += int(nblk_wc[w, ch])
        msg_cols_g.append(off)
    return dict(
        nblk_wc=nblk_wc, nblk_w=nblk_w, NBLK=NBLK, blk_off_wc=blk_off_wc,
        groups=groups, msg_off_gchw=msg_off_gchw, msg_cols_g=msg_cols_g,
        gth=gth, SIDX=sidx,
    )


def build_nc(cap_wc, repeat=1):
    L = _layout(cap_wc)
    NBLK, SIDX = L["NBLK"], L["SIDX"]
    nc = bacc.Bacc("TRN2", target_bir_lowering=False, num_swdge_queues=4)

    x_d = nc.dram_tensor("x", [N_NODES, IN_CH], FP, kind="ExternalInput")
    idx_d = nc.dram_tensor("idx16", [P, SIDX], mybir.dt.int16, kind="ExternalInput")
    rowl_d = nc.dram_tensor("rowl", [P, NBLK], FP, kind="ExternalInput")
    wtsm_d = nc.dram_tensor("wtsm", [P, NBLK], FP, kind="ExternalInput")
    wt_d = nc.dram_tensor("wt", [IN_CH, OUT_CH], FP, kind="ExternalInput")
    bias_d = nc.dram_tensor("bias", [1, OUT_CH], FP, kind="ExternalInput")
    out_d = nc.dram_tensor("out", [NODES_PER_CORE, OUT_CH], FP, kind="ExternalOutput")

    max_msg_cols = max(L["msg_cols_g"])
    max_nblk_w = int(L["nblk_w"].max())

    with tile.TileContext(nc) as tc:
        with (
            tc.tile_pool(name="const", bufs=1) as const_pool,
            tc.tile_pool(name="msg", bufs=5) as msg_pool,
            tc.tile_pool(name="msgb", bufs=4) as msgb_pool,
            tc.tile_pool(name="oh", bufs=8) as oh_pool,
            tc.tile_pool(name="aggp", bufs=2, space="PSUM") as aggp_pool,
            tc.tile_pool(name="aggs", bufs=3) as aggs_pool,
            tc.tile_pool(name="outp", bufs=2, space="PSUM") as outp_pool,
            tc.tile_pool(name="outs", bufs=3) as outs_pool,
        ):
            idx_sb = const_pool.tile([P, SIDX], mybir.dt.int16)
            rowl_sb = const_pool.tile([P, NBLK], FP)
            rowl_bf = const_pool.tile([P, NBLK], BF)
            wtsm_sb = const_pool.tile([P, NBLK], FP)
            iota_sb = const_pool.tile([P, WINDOW], FP)
            iota_bf = const_pool.tile([P, WINDOW], BF)
            wt_sb = const_pool.tile([IN_CH, OUT_CH], FP)
            wt_bf = const_pool.tile([IN_CH, OUT_CH], BF)
            bias_sb = const_pool.tile([1, OUT_CH], FP)
            bias_bf = const_pool.tile([1, OUT_CH], BF)
            ones_bf = const_pool.tile([1, WINDOW], BF)

            nc.sync.dma_start(idx_sb[:], idx_d[:])
            nc.sync.dma_start(rowl_sb[:], rowl_d[:])
            nc.sync.dma_start(wtsm_sb[:], wtsm_d[:])
            nc.sync.dma_start(wt_sb[:], wt_d[:])
            nc.sync.dma_start(bias_sb[:], bias_d[:])
            nc.gpsimd.iota(
                iota_sb[:], pattern=[[1, WINDOW]], base=0,
                channel_multiplier=0, allow_small_or_imprecise_dtypes=True,
            )
            nc.vector.tensor_copy(iota_bf[:], iota_sb[:])
            nc.vector.tensor_copy(rowl_bf[:], rowl_sb[:])
            nc.vector.tensor_copy(wt_bf[:], wt_sb[:])
            nc.vector.tensor_copy(bias_bf[:], bias_sb[:])
            nc.vector.memset(ones_bf[:], 1.0)

            loop_cm = tc.For_i(0, repeat, 1) if repeat > 1 else contextlib.nullcontext()
            with loop_cm:
                for gi, ws in enumerate(L["groups"]):
                    mcols = L["msg_cols_g"][gi]
                    msg = msg_pool.tile([P, max_msg_cols * IN_CH], FP, tag="msg")
                    msgb = msgb_pool.tile([P, max_msg_cols * IN_CH], BF, tag="msgb")
                    # sub-gathers (ring-limited) per (group, chunk)
                    for (gi2, ch, ni, sidx_off, moff) in L["gth"]:
                        if gi2 != gi or ni == 0:
                            continue
                        nblk_g = ni // P
                        nc.gpsimd.dma_gather(
                            out_ap=msg[:, moff * IN_CH : (moff + nblk_g) * IN_CH]
                            .rearrange("p (k d) -> p k d", k=nblk_g),
                            in_ap=x_d[ch * CHUNK : min((ch + 1) * CHUNK, N_NODES), :],
                            idxs_ap=idx_sb[:, sidx_off : sidx_off + ni // 16],
                            num_idxs=ni,
                            num_idxs_reg=ni,
                            elem_size=IN_CH,
                            queue_num=(sidx_off // 56) % 4,
                        )
                    # msgb = msg * w -> bf16 (one batched op per (group, chunk)
                    # region so early chunks feed PE while later chunks gather)
                    gbase = sum(L["msg_cols_g"][:gi])
                    for ch in range(N_CHUNKS):
                        c0 = L["msg_off_gchw"][(ws[0], ch)]
                        ncols = int(sum(L["nblk_wc"][w, ch] for w in ws))
                        if ncols == 0:
                            continue
                        wts_ap = wtsm_sb[:, gbase + c0 : gbase + c0 + ncols]
                        nc.vector.tensor_tensor(
                            out=msgb[:, c0 * IN_CH : (c0 + ncols) * IN_CH]
                            .rearrange("p (k d) -> p k d", k=ncols),
                            in0=msg[:, c0 * IN_CH : (c0 + ncols) * IN_CH]
                            .rearrange("p (k d) -> p k d", k=ncols),
                            in1=bass.AP(
                                wts_ap.tensor, wts_ap.offset,
                                [wts_ap.ap[0], [wts_ap.ap[1][0], ncols], [0, IN_CH]],
                            ),
                            op=mybir.AluOpType.mult,
                        )
                    for w in ws:
                        nblkw = int(L["nblk_w"][w])
                        b0 = int(L["blk_off_wc"][w, 0])
                        # one-hot: eq(iota, rowl_bcast), batched per window, bf16
                        oh = oh_pool.tile([P, max_nblk_w * WINDOW], BF, tag="oh")
                        rowl_ap = rowl_bf[:, b0 : b0 + nblkw]
                        iap = iota_bf[:]
                        nc.vector.tensor_tensor(
                            out=oh[:, : nblkw * WINDOW].rearrange("p (k d) -> p k d", k=nblkw),
                            in0=bass.AP(
                                iap.tensor, iap.offset,
                                [iap.ap[0], [0, nblkw], [iap.ap[1][0], WINDOW]],
                            ),
                            in1=bass.AP(
                                rowl_ap.tensor, rowl_ap.offset,
                                [rowl_ap.ap[0], [rowl_ap.ap[1][0], nblkw], [0, WINDOW]],
                            ),
                            op=mybir.AluOpType.is_equal,
                        )
                        aggT = aggp_pool.tile([IN_CH, WINDOW], FP)
                        jj = 0
                        for ch in range(N_CHUNKS):
                            nblk_c = int(L["nblk_wc"][w, ch])
                            moff = L["msg_off_gchw"][(w, ch)]
                            for j in range(nblk_c):
                                nc.tensor.matmul(
                                    aggT[:],
                                    lhsT=msgb[:, (moff + j) * IN_CH : (moff + j + 1) * IN_CH],
                                    rhs=oh[:, jj * WINDOW : (jj + 1) * WINDOW],
                                    start=(jj == 0),
                                    stop=(jj == nblkw - 1),
                                )
                                jj += 1
                        aggT_sb = aggs_pool.tile([IN_CH, WINDOW], BF)
                        nc.vector.tensor_copy(aggT_sb[:], aggT[:])
                        op = outp_pool.tile([WINDOW, OUT_CH], FP)
                        nc.tensor.matmul(op[:], lhsT=aggT_sb[:], rhs=wt_bf[:], start=True, stop=False)
                        nc.tensor.matmul(op[:], lhsT=ones_bf[:], rhs=bias_bf[:], start=False, stop=True)
                        out_sb = outs_pool.tile([WINDOW, OUT_CH], FP)
                        nc.vector.tensor_copy(out_sb[:], op[:])
                        nc.sync.dma_start(out_d[w * WINDOW : (w + 1) * WINDOW, :], out_sb[:])
    nc.compile()
    return nc


def preprocess(x, edge_index, edge_weight):
    """Bucket edges by (core, window, chunk); pad each bucket to a multiple of
    P (shared across cores). Returns per-core input maps and cap_wc."""
    row = np.asarray(edge_index[0], dtype=np.int64)
    col = np.asarray(edge_index[1], dtype=np.int64)
    wts = np.asarray(edge_weight, dtype=NP_FP)

    gwin = row >> 7                                   # global window id
    ch = col // CHUNK
    key = gwin * N_CHUNKS + ch
    order = np.argsort(key, kind="stable")
    row_s, col_s, w_s, key_s = row[order], col[order], wts[order], key[order]

    n_keys = (N_NODES_PAD >> 7) * N_CHUNKS
    counts = np.bincount(key_s, minlength=n_keys).reshape(
        N_CORES, WINDOWS_PER_CORE, N_CHUNKS
    )
    starts = np.zeros(n_keys + 1, dtype=np.int64)
    np.cumsum(counts.reshape(-1), out=starts[1:])

    cap_wc = -(-counts.max(axis=0) // P) * P          # [W, C] padded counts
    for w in range(WINDOWS_PER_CORE):
        if cap_wc[w].sum() == 0:
            cap_wc[w, 0] = P

    L = _layout(cap_wc)
    NBLK, SIDX = L["NBLK"], L["SIDX"]
    nblk_wc = L["nblk_wc"]

    in_maps = []
    for c in range(N_CORES):
        rowl_a = np.zeros((P, NBLK), dtype=NP_FP)
        wts_blk = np.zeros((P, NBLK), dtype=NP_FP)   # (w, ch, j) block order
        idx_blk = np.zeros((P, NBLK), dtype=np.int16)  # local idx, same order
        for w in range(WINDOWS_PER_CORE):
            for chn in range(N_CHUNKS):
                nblk = int(nblk_wc[w, chn])
                if nblk == 0:
                    continue
                g = c * WINDOWS_PER_CORE + w
                k = g * N_CHUNKS + chn
                s, e = starts[k], starts[k + 1]
                cnt = e - s
                cap = nblk * P
                b0 = int(L["blk_off_wc"][w, chn])
                re_ = np.zeros((cap,), dtype=NP_FP)
                we = np.zeros((cap,), dtype=NP_FP)
                ce = np.zeros((cap,), dtype=np.int16)
                re_[:cnt] = (row_s[s:e] - g * WINDOW).astype(NP_FP)
                we[:cnt] = w_s[s:e]
                ce[:cnt] = (col_s[s:e] - chn * CHUNK).astype(np.int16)
                # edge i -> partition i%128, block i//128
                rowl_a[:, b0 : b0 + nblk] = re_.reshape(nblk, P).T
                wts_blk[:, b0 : b0 + nblk] = we.reshape(nblk, P).T
                idx_blk[:, b0 : b0 + nblk] = ce.reshape(nblk, P).T

        # wtsm: weights in msg-column order ((g, ch, w, j))
        wtsm_a = np.zeros((P, NBLK), dtype=NP_FP)
        # idx16: wrapped indices per (g, ch) gather stream
        idx16 = np.zeros((P, SIDX), dtype=np.int16)
        emitted = set()
        for (gi, chn, ni, sidx_off, moff) in L["gth"]:
            if ni == 0 or (gi, chn) in emitted:
                continue
            emitted.add((gi, chn))
            ni = int(cap_wc[L["groups"][gi], chn].sum())  # full (g,ch) stream
            ws = L["groups"][gi]
            gbase = sum(L["msg_cols_g"][:gi])
            stream = np.zeros((ni,), dtype=np.int16)
            soff = 0
            for w in ws:
                nblk = int(nblk_wc[w, chn])
                if nblk == 0:
                    continue
                b0 = int(L["blk_off_wc"][w, chn])
                cap = nblk * P
                # edges of this bucket in (block, partition) order
                ed_idx = idx_blk[:, b0 : b0 + nblk].T.reshape(-1)
                ed_wts = wts_blk[:, b0 : b0 + nblk].T.reshape(-1)
                stream[soff : soff + cap] = ed_idx
                mo = gbase + L["msg_off_gchw"][(w, chn)]
                wtsm_a[:, mo : mo + nblk] = ed_wts.reshape(nblk, P).T
                soff += cap
            # wrap: idx i -> partition i%16, column i//16; replicate 8x
            wrapped = stream.reshape(ni // 16, 16).T    # [16, ni/16]
            idx16[:, sidx_off : sidx_off + ni // 16] = np.tile(wrapped, (8, 1))

        in_maps.append({"idx16": idx16, "rowl": rowl_a, "wtsm": wtsm_a})
    return in_maps, cap_wc


_CACHE = {}


def finalize_in_maps(in_maps, x, W, b):
    """Attach shared tensors (x, weights, bias)."""
    x = np.asarray(x, dtype=NP_FP)
    wt = np.ascontiguousarray(np.asarray(W, dtype=NP_FP).T)
    bias = np.asarray(b, dtype=NP_FP).reshape(1, OUT_CH)
    for c in range(N_CORES):
        in_maps[c]["x"] = x
        in_maps[c]["wt"] = wt
        in_maps[c]["bias"] = bias
    return in_maps


def kernel(x, edge_index, edge_weight, W, b):
    x = np.asarray(x, dtype=NP_FP)

    in_maps, cap_wc = preprocess(x, edge_index, edge_weight)

    key = cap_wc.tobytes()
    if key not in _CACHE:
        _CACHE[key] = build_nc(cap_wc)
    nc = _CACHE[key]

    finalize_in_maps(in_maps, x, W, b)

    res = run_bass_kernel_spmd(nc, in_maps, core_ids=list(range(N_CORES)))
    out = np.concatenate([res.results[c]["out"] for c in range(N_CORES)], axis=0)
    return out[:N_NODES]
